# revision 61
# baseline (speedup 1.0000x reference)
"""Trainium2 Bass kernel for nn_MultiHeadAttention_65773129171319 (v3).

Complex-valued multi-head attention:
  attn = softmax(|Qc Kc^H| / sqrt(2 dk)) ; out = (attn @ Vr) Wo, (attn @ Vp) Wo

Sharding: 8 cores = 2 (batch) x 4 (head-groups of 2 heads).  Each core
computes its batch's full sequence for its 2 heads; the out-projection
partial sums (over head groups) are reduced on the host.

Device algorithm (per core, all matmuls bf16, fp32 PSUM):
  - Q/K projections run at full PE width (M=128 = both heads); the
    per-head stacked score operands
      qc_h  = [Qr_h ; Qp_h],  kcr_h = [Kr_h ; -Kp_h],  kcp_h = [Kp_h ; Kr_h]
    are assembled with aligned engine copies + 2 partition-crossing
    SBUF->SBUF DMAs per strip (DMA is the only partition-crossing path).
  - Scores (transposed [sk, sq]) via single 128-contraction matmuls;
    strip 0's score tiles are emitted inside the A-phase as soon as the
    kc columns they need exist, hiding the projection/DMA prologue.
  - |z| = sqrt(r^2+p^2) is an 8-stage fused DVE op (no ACT sqrt pass, no
    table switching):
      m = max(C0*s + C1*d, C2*s),  s = |r|+|p|, d = ||r|-|p||
    the optimal 2-piece max-affine approx on the folded quarter-circle;
    max rel err 0.97% after exp's scale absorbs a global factor.  |p|
    comes from an ACT abs-copy (abs is in every ACT table set, so the
    only table load is exp's, forced first by a dummy exp).
  - attn = exp(scale*m) streams per tile right behind the maxaff -- 2/3
    of tiles on ACT (paired 1024-wide), 1/3 on DVE as (q(m)^2)^2 with a
    quadratic q (max rel err 1.2e-3) to balance ACT vs DVE occupancy.
  - AV uses a stacked stationary [Vr_h | Vp_h]: out partitions 0:63 are
    the real part, 64:127 the phase part -> one matmul per tile.
  - Softmax denominators: ones-stationary rowsum matmuls, fast DVE
    reciprocal, GPSIMD partition-broadcast, fused into the PSUM->SBUF
    normalisation multiplies; consumer stages lag the score stream by a
    strip so no engine FIFO ever waits on a fresh exp.
  - Out-projection per strip: full 128-contraction per sq tile.
"""

import os
import sys

import numpy as np

try:
    import concourse.bass as bass
except ImportError:  # pragma: no cover
    sys.path.insert(0, "/opt/trn_rl_repo")
    import concourse.bass as bass

import ml_dtypes
import concourse.mybir as mybir
import concourse.tile as tile
from concourse import bacc
from concourse.bass_utils import run_bass_kernel_spmd

B, S, D, H = 2, 2048, 512, 8
DK = D // H  # 64
SCALE = float((2 * DK) ** 0.5)
P = 128
N_CORES = 8
HG = 4            # head groups (2 heads each)
DT = D // P       # 4 d-tiles for projection contraction
SKT = S // P      # 16 sk tiles
NSTRIP = 4        # sq strips of 512
STRIP = S // NSTRIP  # 512

F32 = mybir.dt.float32
BF16 = mybir.dt.bfloat16
BFNP = ml_dtypes.bfloat16

AF = mybir.ActivationFunctionType

# 2-piece max-affine approx of sqrt(mx^2+mn^2) on the folded domain:
#   c*max(1 + b1*t, g*(1+t)), t = mn/mx in [0,1]
_B1 = 0.2679403
_G = 0.73205046
_C = 0.98266811
CS = (1.0 + _B1) / 2.0      # coefficient of s = |r|+|p|
CD = (1.0 - _B1) / 2.0      # coefficient of d = ||r|-|p||
CG = _G
ESCALE = _C / SCALE          # exp scale absorbs the global factor c


def register_custom_ops():
    """Register the fused |z|-approx DVE op (runtime extension of OPS)."""
    import concourse.dve_ops as dve_ops
    from concourse.dve_ops import DveOp
    from concourse.dve_spec import (Spec, Src0, Src1, Bin, Zero,
                                    C0, C1, C2, maxx, lower, _has_src1)
    from concourse.dve_uop import AluOp, DveOpSpec

    existing = {op.name: op for op in dve_ops.OPS}

    def mk(name, spec):
        if name in existing:
            return existing[name]
        row = max(dve_ops._SUB_OPCODE_FOR_NAME.values()) + 1
        assert row < 0x20, "no free DVE opcode rows"
        dve_ops._SUB_OPCODE_FOR_NAME[name] = row
        shas = {}
        for ver in ("v3", "v4"):
            s = DveOpSpec(name=name, opcode=row, uops=lower(spec, ver=ver),
                          rd1_en=_has_src1(spec))
            shas[ver] = s.sha(ver)
        op = DveOp(name, spec, subdim=False, uops_sha=shas)
        dve_ops.OPS.append(op)
        return op

    def _ref_maxaff(in0, in1, s0, s1, imm2):
        a = np.abs(in0.astype(np.float32))
        b = in1.astype(np.float32)
        ss = a + b
        dd = np.abs(a - b)
        return np.maximum(np.float32(s0) * ss + np.float32(s1) * dd,
                          np.float32(imm2) * ss)

    ar = Bin(AluOp.ABSOLUTE_DIFF, Src0, Zero)
    s_ = ar + Src1
    d_ = Bin(AluOp.ABSOLUTE_DIFF, ar, Src1)
    maxaff = mk("MAXAFF_ANT", Spec(
        body=maxx(C0 * s_ + C1 * d_, C2 * s_),
        reference=_ref_maxaff))
    abscopy = mk("ABSCOPY_ANT", Spec(
        body=Bin(AluOp.ABSOLUTE_DIFF, Src0, Zero),
        reference=lambda in0, in1, s0, s1, imm2: np.abs(in0.astype(np.float32))))

    def _ref_exppoly(in0, in1, s0, s1, imm2):
        x = in0.astype(np.float32)
        q = (np.float32(s0) + x * (np.float32(s1) + np.float32(imm2) * x))
        return (q * q) ** 2

    from concourse.dve_spec import sq
    q_ = C0 + Src0 * (C1 + C2 * Src0)
    exppoly = mk("EXPPOLY_ANT", Spec(
        body=sq(sq(q_)),
        reference=_ref_exppoly))
    return maxaff, abscopy, exppoly


MAXAFF, ABSCOPY, EXPPOLY = register_custom_ops()

# quadratic fit of exp(ESCALE*m/4) on m in [0,18]; attn = q^4
EC0, EC1, EC2 = 1.00031029, 2.13791850e-02, 2.85949774e-04


def build(n_iter: int = 1, variant: frozenset = frozenset()):
    """Build (and bacc-compile) the per-core SPMD program."""
    nc = bacc.Bacc("TRN2", target_bir_lowering=False, debug=False,
                   num_devices=N_CORES)

    dr = {}
    for name in ("xqr", "xqp", "xkr", "xkp", "xvr", "xvp"):
        dr[name] = nc.dram_tensor(name, [D, S], BF16, kind="ExternalInput")
    for name in ("wq", "wk", "wv"):
        dr[name] = nc.dram_tensor(name, [D, 2 * DK], BF16, kind="ExternalInput")
    dr["wo"] = nc.dram_tensor("wo", [2 * DK, D], BF16, kind="ExternalInput")
    dr["o_r"] = nc.dram_tensor("o_r", [S, D], F32, kind="ExternalOutput")
    dr["o_p"] = nc.dram_tensor("o_p", [S, D], F32, kind="ExternalOutput")

    with tile.TileContext(nc) as tc:
        _emit(tc, dr, n_iter, variant)
    nc.compile()
    return nc


def _emit(tc, dr, n_iter, variant=frozenset()):
    from contextlib import ExitStack

    ctx = ExitStack()
    with ctx:
        pools = dict(
            singles=ctx.enter_context(tc.tile_pool(name="singles", bufs=1)),
            xpool=ctx.enter_context(tc.tile_pool(name="xp", bufs=6)),
            projpool=ctx.enter_context(tc.tile_pool(name="pj", bufs=2)),
            ppool=ctx.enter_context(tc.tile_pool(name="pp", bufs=8)),
            apool=ctx.enter_context(tc.tile_pool(name="ap", bufs=4)),
            kcpool=ctx.enter_context(tc.tile_pool(name="kcp", bufs=2)),
            tpool=ctx.enter_context(tc.tile_pool(name="tp", bufs=3)),
            opool=ctx.enter_context(tc.tile_pool(name="op", bufs=3)),
            psA=ctx.enter_context(tc.tile_pool(name="psA", bufs=6, space="PSUM")),
            psAV=ctx.enter_context(tc.tile_pool(name="psAV", bufs=2, space="PSUM")),
        )
        nc = tc.nc
        singles = pools["singles"]
        wsb = {}
        for name in ("wq", "wk", "wv"):
            t = singles.tile([P, DT, 2 * DK], BF16, tag=f"w_{name}", name=f"w_{name}")
            nc.sync.dma_start(out=t[:], in_=dr[name].rearrange("(dt p) m -> p dt m", p=P))
            wsb[name] = t
        wo = singles.tile([P, D], BF16, tag="w_wo", name="w_wo")
        nc.sync.dma_start(out=wo[:], in_=dr["wo"][:])
        ones = singles.tile([P, 1], BF16, tag="ones", name="ones")
        nc.vector.memset(ones[:], 1.0)
        # force exp_and_others to be the (only) loaded ACT table set
        dmy = singles.tile([1, 8], F32, tag="dmy", name="dmy")
        nc.vector.memset(dmy[:], 0.0)
        nc.scalar.activation(dmy[:], dmy[:], AF.Exp)
        consts = (wsb, wo, ones)
        if n_iter > 1 and "unroll" not in variant:
            with tc.For_i(0, n_iter, 1):
                _body(tc, dr, variant, consts, **pools)
        else:
            for _ in range(max(1, n_iter if "unroll" in variant else 1)):
                _body(tc, dr, variant, consts, **pools)


def _body(tc, dr, variant, consts, singles, xpool, projpool, ppool, apool,
          tpool, opool, kcpool, psA, psAV):
    nc = tc.nc
    wsb, wo, ones = consts

    # ---- per-iteration SBUF tensors (kc/vstk double-buffered across the
    # timing loop so iteration i+1's projections overlap iteration i's
    # attention tail) ------------------------------------------------------
    # kc layout: (qc0, kcr0, kcp0, qc1, kcr1, kcp1); qc_h=[Qr;Qp],
    # kcr_h=[Kr;-Kp], kcp_h=[Kp;Kr] stacked on partitions (64+64).
    kc = kcpool.tile([P, 6, S], BF16, tag="kc", name="kc")
    # vstk layout [sk_part, t, h, rp*dk]: AV stationary for (t,h) is
    # vstk[:, t, h, :] = [Vr_h | Vp_h] (128 contiguous cols).
    vstk = kcpool.tile([P, SKT, 2, 2 * DK], BF16, tag="vstk", name="vstk")
    xr = singles.tile([P, S], BF16, tag="xr", name="xr")
    xp = singles.tile([P, S], BF16, tag="xp", name="xp")
    xstg = singles.tile([P, S], BF16, tag="xstg", name="xstg")


    attn_of = {}
    rs_of = {}
    CONS_LAG = 1

    def score_tiles(s, h, trange):
        ssl = slice(s * STRIP, (s + 1) * STRIP)
        attn = attn_of[(s, h)]
        for t in trange:
            tsl = slice(t * P, (t + 1) * P)
            ps_r = psA.tile([P, STRIP], F32, tag="psA", name="psA")
            nc.tensor.matmul(ps_r[:], kc[:, 3 * h + 1, tsl],
                             kc[:, 3 * h, ssl], start=True, stop=True)
            ps_p = psA.tile([P, STRIP], F32, tag="psA", name="psA")
            nc.tensor.matmul(ps_p[:], kc[:, 3 * h + 2, tsl],
                             kc[:, 3 * h, ssl], start=True, stop=True)
            if "nopost" in variant:
                continue
            pa = ppool.tile([P, STRIP], BF16, tag="pa", name="pa")
            nc.scalar.activation(pa[:], ps_p[:], AF.Abs)
            nc.vector._custom_dve(MAXAFF, out=attn[:, t, :], in0=ps_r[:],
                                  in1=pa[:], s0=CS, s1=CD, imm2=CG)
            if "noexp" not in variant:
                if (t // 2) % 3 == 2:
                    nc.vector._custom_dve(EXPPOLY, out=attn[:, t, :],
                                          in0=attn[:, t, :],
                                          s0=EC0, s1=EC1, imm2=EC2)
                elif t % 2 == 1:
                    nc.scalar.activation(attn[:, t - 1:t + 1, :],
                                         attn[:, t - 1:t + 1, :],
                                         AF.Exp, scale=ESCALE)

    def new_attn(s):
        for h in range(2):
            attn = apool.tile([P, SKT, STRIP], BF16, tag="attn", name="attn")
            attn_of[(s, h)] = attn
            if "nopost" in variant:
                nc.vector.memset(attn[:], 0.25)


    # ---- A phase: Q/K/V projections + kc assembly ------------------------
    # proj_sb order: (Qr, Kr, Kp, | Qp, Kpn, Kr2).  Aligned halves copy
    # straight; the two partition-crossing groups go via SBUF->SBUF DMA:
    #   cross A: kc[64:,(0,1,2)] <- proj_sb[0:64,(3,4,5)]   (Qp, Kpn, Kr)
    #   cross B: kc[0:64,(3,4,5)] <- proj_sb[64:,(0,1,2)]   (Qr, Kr, Kp)
    # Staging copies ride the ACT engine (idle during this phase).
    for s in range(NSTRIP):
        if "noA" in variant:
            break
        ssl = slice(s * STRIP, (s + 1) * STRIP)
        xs = {}
        for nm in ("xqr", "xqp", "xkr", "xkp", "xvr", "xvp"):
            t = xpool.tile([P, DT, STRIP], BF16, tag="xs", name="xs")
            nc.sync.dma_start(out=t[:], in_=dr[nm].rearrange(
                "(dt p) s -> p dt s", p=P)[:, :, ssl])
            xs[nm] = t
        proj = projpool.tile([P, 6, STRIP], BF16, tag="proj", name="proj")
        for idx, (xnm, wnm) in enumerate(
                (("xqr", "wq"), ("xkr", "wk"), ("xkp", "wk"), ("xqp", "wq"))):
            ps = psA.tile([P, STRIP], F32, tag="psA", name="psA")
            for dt in range(DT):
                nc.tensor.matmul(ps[:], wsb[wnm][:, dt, :], xs[xnm][:, dt, :],
                                 start=(dt == 0), stop=(dt == DT - 1))
            if idx == 0:      # Qr -> 0
                nc.scalar.copy(proj[:, 0, :], ps[:])
            elif idx == 1:    # Kr -> 1 and 5
                nc.scalar.copy(proj[:, 1, :], ps[:])
                nc.scalar.copy(proj[:, 5, :], ps[:])
            elif idx == 2:    # Kp -> 2, Kpn -> 4
                nc.scalar.copy(proj[:, 2, :], ps[:])
                nc.scalar.mul(proj[:, 4, :], ps[:], -1.0)
            else:             # Qp -> 3
                nc.scalar.copy(proj[:, 3, :], ps[:])
        # aligned halves
        nc.scalar.copy(kc[0:DK, 0:3, ssl], proj[0:DK, 0:3, :])
        nc.scalar.copy(kc[DK:P, 3:6, ssl], proj[DK:P, 3:6, :])
        # partition-crossing halves
        nc.sync.dma_start(out=kc[DK:P, 0:3, ssl], in_=proj[0:DK, 3:6, :])
        nc.sync.dma_start(out=kc[0:DK, 3:6, ssl], in_=proj[DK:P, 0:3, :])
        # V projection for this strip
        if "noV" in variant:
            continue
        for rp in (0, 1):
            for tt in range(STRIP // P):
                t = s * (STRIP // P) + tt
                psv = psAV.tile([P, 2 * DK], F32, tag="av", name="av")
                xv = xs["xvr"] if rp == 0 else xs["xvp"]
                for dt in range(DT):
                    nc.tensor.matmul(psv[:], xv[:, dt, tt * P:(tt + 1) * P],
                                     wsb["wv"][:, dt, :],
                                     start=(dt == 0), stop=(dt == DT - 1))
                nc.vector.tensor_copy(
                    vstk[:, t, :, rp * DK:(rp + 1) * DK],
                    psv[:].rearrange("p (h k) -> p h k", h=2))
        # strip-0 attention tiles whose kcr/kcp columns this A-strip provides
        if "noscores" not in variant:
            if s == 0:
                new_attn(0)
            for h in range(2):
                score_tiles(0, h, range(4 * s, 4 * s + 4))

    # ---- attention + out-projection per strip ---------------------------
    # Emission order per strip s: scores+abs+maxaff(s) -> exps(s) ->
    # consumers(s-1) -> out-proj(s-2).  The one-strip consumer lag keeps
    # the PE from waiting on exp; the two-strip out-proj lag keeps it
    # from waiting on the partition-crossing DMA.
    def consumers(s):
        ssl = slice(s * STRIP, (s + 1) * STRIP)
        for h in range(2):
            attn = attn_of[(s, h)]
            ps_rs = psAV.tile([1, STRIP], F32, tag="av", name="av")
            for t in range(SKT):
                nc.tensor.matmul(ps_rs[0:1, :], ones[:], attn[:, t, :],
                                 start=(t == 0), stop=(t == SKT - 1))
            rrec = tpool.tile([1, STRIP], F32, tag="rrec", name="rrec")
            nc.vector.reciprocal_approx_fast(rrec[:], ps_rs[0:1, :])
            rb = tpool.tile([P, STRIP], F32, tag="rb", name="rb")
            nc.gpsimd.partition_broadcast(rb[:], rrec[:])
            ps_av = psAV.tile([P, STRIP], F32, tag="av", name="av")
            for t in range(SKT):
                nc.tensor.matmul(ps_av[:], vstk[:, t, h, :], attn[:, t, :],
                                 start=(t == 0), stop=(t == SKT - 1))
            if h == 0:
                nc.vector.tensor_mul(xr[0:DK, ssl], ps_av[0:DK, :], rb[0:DK, :])
                nc.vector.tensor_mul(xstg[DK:P, ssl], ps_av[DK:P, :], rb[DK:P, :])
            else:
                nc.vector.tensor_mul(xstg[0:DK, ssl], ps_av[0:DK, :], rb[0:DK, :])
                nc.vector.tensor_mul(xp[DK:P, ssl], ps_av[DK:P, :], rb[DK:P, :])
        # h1's real part and h0's phase part cross partition halves via DMA
        nc.sync.dma_start(out=xr[DK:P, ssl], in_=xstg[0:DK, ssl])
        nc.sync.dma_start(out=xp[0:DK, ssl], in_=xstg[DK:P, ssl])

    def outproj(s):
        if "noout" in variant:
            return
        for q in range(STRIP // P):
            qsl = slice(s * STRIP + q * P, s * STRIP + (q + 1) * P)
            for xT, out in ((xr, dr["o_r"]), (xp, dr["o_p"])):
                ps_o = psA.tile([P, D], F32, tag="psA", name="psA")
                nc.tensor.matmul(ps_o[:], xT[:, qsl], wo[:], start=True, stop=True)
                osb = opool.tile([P, D], F32, tag="osb", name="osb")
                nc.vector.tensor_copy(osb[:], ps_o[:])
                nc.sync.dma_start(out=out[qsl, :], in_=osb[:])

    for s in range(1, NSTRIP):
        if "noscores" in variant:
            break
        new_attn(s)
        for h in range(2):
            score_tiles(s, h, range(SKT))
        if "nocons" in variant:
            continue
        if s >= CONS_LAG:
            consumers(s - CONS_LAG)
            outproj(s - CONS_LAG)
    if "nocons" not in variant and "noscores" not in variant:
        for s in range(NSTRIP - CONS_LAG, NSTRIP):
            consumers(s)
            outproj(s)


# ---------------------------------------------------------------------------
_CACHE = {}


def _get_nc(n_iter=1, variant=frozenset()):
    key = (n_iter, variant)
    if key not in _CACHE:
        _CACHE[key] = build(n_iter, variant)
    return _CACHE[key]


def make_in_maps(q_real, k_real, v_real, q_phase, k_phase, v_phase,
                 w_q, w_k, w_v, w_o):
    """Host-side shard + layout prep: per-core input dicts."""
    xt = {}
    for b in range(B):
        xt[("xqr", b)] = np.ascontiguousarray(q_real[b].T).astype(BFNP)
        xt[("xqp", b)] = np.ascontiguousarray(q_phase[b].T).astype(BFNP)
        xt[("xkr", b)] = np.ascontiguousarray(k_real[b].T).astype(BFNP)
        xt[("xkp", b)] = np.ascontiguousarray(k_phase[b].T).astype(BFNP)
        xt[("xvr", b)] = np.ascontiguousarray(v_real[b].T).astype(BFNP)
        xt[("xvp", b)] = np.ascontiguousarray(v_phase[b].T).astype(BFNP)
    wq16, wk16, wv16, wo16 = (w.astype(BFNP) for w in (w_q, w_k, w_v, w_o))
    in_maps = []
    for core in range(N_CORES):
        b, hg = divmod(core, HG)
        csl = slice(hg * 2 * DK, (hg + 1) * 2 * DK)
        in_maps.append({
            "xqr": xt[("xqr", b)], "xqp": xt[("xqp", b)],
            "xkr": xt[("xkr", b)], "xkp": xt[("xkp", b)],
            "xvr": xt[("xvr", b)], "xvp": xt[("xvp", b)],
            "wq": np.ascontiguousarray(wq16[:, csl]),
            "wk": np.ascontiguousarray(wk16[:, csl]),
            "wv": np.ascontiguousarray(wv16[:, csl]),
            "wo": np.ascontiguousarray(wo16[csl, :]),
        })
    return in_maps


def gather_outputs(results):
    out_r = np.zeros((B, S, D), np.float32)
    out_p = np.zeros((B, S, D), np.float32)
    for core in range(N_CORES):
        b = core // HG
        out_r[b] += np.asarray(results[core]["o_r"], np.float32)
        out_p[b] += np.asarray(results[core]["o_p"], np.float32)
    return out_r, out_p


def _numpy_fallback(q_real, k_real, v_real, q_phase, k_phase, v_phase,
                    w_q, w_k, w_v, w_o, mask):
    def heads(x, w):
        y = x @ w
        return y.reshape(B, -1, H, DK).transpose(0, 2, 1, 3)
    qr, kr, vr = heads(q_real, w_q), heads(k_real, w_k), heads(v_real, w_v)
    qp, kp, vp = heads(q_phase, w_q), heads(k_phase, w_k), heads(v_phase, w_v)
    ar = np.einsum('bhqd,bhkd->bhqk', qr, kr) - np.einsum('bhqd,bhkd->bhqk', qp, kp)
    ap = np.einsum('bhqd,bhkd->bhqk', qr, kp) + np.einsum('bhqd,bhkd->bhqk', qp, kr)
    a = np.sqrt(ar * ar + ap * ap) / SCALE
    a = np.where(mask[:, None, :, :] == 0, np.float32(-1e9), a)
    a = a - a.max(axis=-1, keepdims=True)
    e = np.exp(a)
    a = e / e.sum(axis=-1, keepdims=True)
    xr = np.einsum('bhqk,bhkd->bhqd', a, vr).transpose(0, 2, 1, 3).reshape(B, -1, D)
    xp = np.einsum('bhqk,bhkd->bhqd', a, vp).transpose(0, 2, 1, 3).reshape(B, -1, D)
    return (xr @ w_o).astype(np.float32), (xp @ w_o).astype(np.float32)


def kernel(q_real, k_real, v_real, q_phase, k_phase, v_phase,
           w_q, w_k, w_v, w_o, mask):
    args = [np.asarray(a, np.float32) for a in
            (q_real, k_real, v_real, q_phase, k_phase, v_phase,
             w_q, w_k, w_v, w_o)]
    mask = np.asarray(mask)
    if not np.all(mask != 0):
        return _numpy_fallback(*args, mask)
    nc = _get_nc(1)
    in_maps = make_in_maps(*args)
    res = run_bass_kernel_spmd(nc, in_maps, core_ids=list(range(N_CORES)))
    return gather_outputs(res.results)
